# revision 41
# baseline (speedup 1.0000x reference)
"""CrossTableAttention Trainium2 kernel v4 (8-core SPMD, batch-sharded).

Math (per table t, row b, head h, relation slot s):
  rw[t,r]   = sigmoid(rel_embs[t,r] . w_rel + b_rel)             (host)
  qT[f,tb]  = (emb @ Wq.T).T  (feature-on-partition, dh-major perm)
  kT, vT    likewise (bk softmax-invariant -> dropped; bv folds to host
              output addend Wo@bv since sum_r attn = 1)
  sc[(s,h), b] = 0.125*rw * sum_f qT kT     (DVE mul+folds, then one PE
              matmul per (t,j) pair with zero-padded selection stationary,
              accumulating straight into a per-table PSUM tile scT)
  P~ = exp(sc + ln rw)        (one ACT op per table, bias-folded rw)
  Z[h,b] = sum_s exp(sc)      (tiny PE matmul vs 1/rw selection weights)
  ctx_unnorm[f,b] = sum_pairs (merged P~)(h(f),b) * vT[f,jb]   (DVE)
  ctx = ctx_unnorm * (1/Z)[h(f),b]   (replicated via tiny PE matmul)
  out = (ctx.T @ Wo.T).T

v4 structure (from v3 trace: DVE idle first 90us, PE starved 75us mid-run,
both engines ~70% busy over a 367us span vs ~250us of work each):
 - q/k projections interleaved at column-QUARTER granularity so the first
   score pairs are ready ~40us in; v follows; o fills the PE tail.
 - scores land transposed [(s,h), b] in a per-table PSUM tile built by
   per-pair accumulating matmuls (shared-bank start-flag clears); kills
   the per-table PE transpose and the softmax serialization of v3.
 - softmax: one Exp ACT per table with ln(rw) bias (P~ = rw*e in one op),
   Z via a 16-col matmul against 1/rw selection weights, reciprocal on
   DVE from SBUF, replication to 128 partitions via a tiny PE matmul.
 - ctx accumulates into qT's SBUF block per table (dead exactly then),
   normalized once per table; o-projection per 4-table group streams out
   as soon as its ctx blocks normalize.
"""

import sys

sys.path.insert(0, "/opt/trn_rl_repo")

import numpy as np
import ml_dtypes

import concourse.bass as bass
import concourse.bacc as bacc_mod
import concourse.mybir as mybir
import concourse.tile as tile
from concourse.bass_utils import run_bass_kernel_spmd

T, B, D, R, H = 16, 1024, 1024, 8, 16
DH = D // H  # 64
NCORES = 8
BC = B // NCORES  # 128 rows per core
KCH = D // 128  # 8 feature chunks
TB = T * BC  # 2048 (t,b) columns per core
NQ = 4  # column quarters for q/k
QW = TB // NQ  # 512 cols per quarter

F32 = mybir.dt.float32
BF16 = mybir.dt.bfloat16
AF = mybir.ActivationFunctionType

# feature permutation: new f = dh*16 + h  <->  old o = h*64 + dh
_PERM = np.array([(f % H) * DH + f // H for f in range(D)], dtype=np.int64)


def _bcast_free(ap, n, pos):
    """Insert a [step=0, n] broadcast dim into an AP's free dims at `pos`."""
    new = list(ap.ap)
    new.insert(1 + pos, [0, n])
    return bass.AP(tensor=ap.tensor, offset=ap.offset, ap=new)


def _structure(rel_idx):
    """Slot assignment: per t, unique j's get consecutive slot runs."""
    pairs = []  # (t, j, s0, m)
    slot_r = np.zeros((T, R), np.int64)
    for t in range(T):
        by_j = {}
        for r in range(R):
            by_j.setdefault(int(rel_idx[t, r]), []).append(r)
        s0 = 0
        for j, rs in sorted(by_j.items()):
            pairs.append((t, j, s0, len(rs)))
            for i, r in enumerate(rs):
                slot_r[t, s0 + i] = r
            s0 += len(rs)
    combos = sorted({(s0, m) for (_, _, s0, m) in pairs})
    merge_idx = {c: i for i, c in enumerate(combos)}
    return pairs, slot_r, merge_idx


def _pad_window(s0, m):
    """Smallest (b0, w) slot window covering [s0, s0+m) with a 32-aligned
    partition base and legal tile col size: w=2 -> b0 in {0,2,4,6};
    w=4 -> {0,4}; w>=6 -> b0=0."""
    for w, bases in ((2, (0, 2, 4, 6)), (4, (0, 4)), (6, (0,)), (8, (0,))):
        for b0 in bases:
            if b0 <= s0 and b0 + w >= s0 + m:
                return b0, w
    raise AssertionError((s0, m))


def _build(rel_idx, use_bq, use_bo):
    pairs, slot_r, merge_idx = _structure(rel_idx)
    ncmb = len(merge_idx)
    # selp packing offsets
    sel_off = {}
    off = 0
    pad = {}
    for i, (t, j, s0, m) in enumerate(pairs):
        b0, w = _pad_window(s0, m)
        pad[i] = (b0, w)
        sel_off[i] = off
        off += w * 16
    sel_cols = off

    nc = bacc_mod.Bacc(None, target_bir_lowering=False, debug=False)
    emb_ext = nc.dram_tensor("emb", [KCH, 128, TB], BF16, kind="ExternalInput")
    wq_ext = nc.dram_tensor("wq", [D, D], BF16, kind="ExternalInput")
    wk_ext = nc.dram_tensor("wk", [D, D], BF16, kind="ExternalInput")
    wv_ext = nc.dram_tensor("wv", [D, D], BF16, kind="ExternalInput")
    wo_ext = nc.dram_tensor("wo", [D, D], BF16, kind="ExternalInput")
    selp_ext = nc.dram_tensor("selp", [128, sel_cols], BF16, kind="ExternalInput")
    ones_ext = nc.dram_tensor("ones16", [128, 16], BF16, kind="ExternalInput")
    rwrep_ext = nc.dram_tensor("rwrep", [128, T], BF16, kind="ExternalInput")
    merge_ext = nc.dram_tensor("mrg", [128, ncmb * 128], BF16, kind="ExternalInput")
    repl_ext = nc.dram_tensor("repl", [16, 128], F32, kind="ExternalInput")
    if use_bq:
        bq_ext = nc.dram_tensor("bqp", [128, KCH], F32, kind="ExternalInput")
    if use_bo:
        bo_ext = nc.dram_tensor("boe", [128, KCH], F32, kind="ExternalInput")
    out_ext = nc.dram_tensor("out", [KCH, 128, TB], F32, kind="ExternalOutput")

    with tile.TileContext(nc) as tc:
        with (
            # PSUM budget (8 banks): scT 4 + work 3 + chain 1
            tc.tile_pool(name="scps", bufs=1, space="PSUM") as scps,
            tc.tile_pool(name="workps", bufs=3, space="PSUM") as workps,
            tc.tile_pool(name="chainps", bufs=1, space="PSUM") as chainps,
            tc.tile_pool(name="consts", bufs=1) as consts,
            tc.tile_pool(name="wpool", bufs=2) as wpool,
            tc.tile_pool(name="embp", bufs=1) as embp,
            tc.tile_pool(name="qp", bufs=1) as qp,
            tc.tile_pool(name="kp", bufs=1) as kp,
            tc.tile_pool(name="vp", bufs=1) as vp,
            tc.tile_pool(name="pt", bufs=1) as ptp,
            tc.tile_pool(name="prodp", bufs=3) as prodp,
            tc.tile_pool(name="pbsb", bufs=16) as pbsbp,
            tc.tile_pool(name="zsb", bufs=1) as zsbp,
            tc.tile_pool(name="outp", bufs=2) as outp,
        ):
            # ---- PSUM tiles ----
            scT = scps.tile([128, T, 128], F32)  # 4 banks, per-table scores
            # Pre-zero scT: score matmuls then accumulate with start=False
            # (value-correct for both stale has_written states). A start=True
            # bank-clear MM is unusable here: the scheduler may reorder it
            # after sibling tables' disjoint-region MMs, wiping them.
            nc.vector.memset(scT[:], 0.0)
            # ---- weights first (gate the first csteps), consts off-queue ----
            wq_t = wpool.tile([128, KCH, D], BF16, tag="w")
            nc.gpsimd.dma_start(
                out=wq_t, in_=wq_ext.rearrange("(k p) o -> p k o", p=128)
            )
            wk_t = wpool.tile([128, KCH, D], BF16, tag="w")
            nc.gpsimd.dma_start(
                out=wk_t, in_=wk_ext.rearrange("(k p) o -> p k o", p=128)
            )

            # embT[p, k, tb] — host pre-transposed; quarter-ordered loads
            embT = embp.tile([128, KCH, TB], BF16)
            for qtr in range(NQ):
                for k in range(KCH):
                    eng = nc.sync
                    eng.dma_start(
                        out=embT[:, k, qtr * QW : (qtr + 1) * QW],
                        in_=emb_ext[k, :, qtr * QW : (qtr + 1) * QW],
                    )

            selp = consts.tile([128, sel_cols], BF16)
            nc.gpsimd.dma_start(out=selp, in_=selp_ext[:])
            ones_sel = consts.tile([128, 16], BF16)
            nc.gpsimd.dma_start(out=ones_sel, in_=ones_ext[:])
            rwrep = consts.tile([128, T], BF16)
            nc.gpsimd.dma_start(out=rwrep, in_=rwrep_ext[:])
            mergeT = consts.tile([128, ncmb * 128], BF16)
            nc.gpsimd.dma_start(out=mergeT, in_=merge_ext[:])
            replpat = consts.tile([16, 128], F32)
            nc.gpsimd.dma_start(out=replpat, in_=repl_ext[:])
            if use_bq:
                bqp = consts.tile([128, KCH], F32)
                nc.gpsimd.dma_start(out=bqp, in_=bq_ext[:])
            if use_bo:
                boe = consts.tile([128, KCH], F32)
                nc.gpsimd.dma_start(out=boe, in_=bo_ext[:])

            qT = qp.tile([128, KCH, TB], BF16)  # becomes ctx per t-block
            kT = kp.tile([128, KCH, TB], BF16)
            vT = vp.tile([128, KCH, TB], BF16)
            Pt = ptp.tile([128, T, 128], BF16)  # P~ per table
            rzrepS = ptp.tile([128, T, 128], BF16)  # 1/Z replicated per table

            # ---------- emit helpers ----------
            def proj_qstep(w_t, dst_all, qtr, c, bias_t, dve_copy=False):
                """quarter cstep: 8 k-matmuls (N=512) + PSUM->SBUF copy."""
                ps = workps.tile([128, QW], F32, tag="pp", name="pp")
                base = qtr * QW
                for k in range(KCH):
                    nc.tensor.matmul(
                        ps,
                        w_t[:, k, c * 128 : (c + 1) * 128],
                        embT[:, k, base : base + QW],
                        start=(k == 0),
                        stop=(k == KCH - 1),
                    )
                d = dst_all[:, c, base : base + QW]
                if bias_t is not None:
                    nc.scalar.activation(d, ps, AF.Identity, bias=bias_t[:, c])
                elif dve_copy:
                    nc.vector.tensor_copy(out=d, in_=ps)
                else:
                    nc.scalar.copy(out=d, in_=ps)

            bank_left = [0] * 4
            for (t, j, s0, m) in pairs:
                bank_left[t // 4] += 1

            def _score_mm(i, prod, poff, nk):
                """score matmul; moving operand covers nk 128-col chunks of
                the (partially folded) product; the output AP aliases the
                chunk dim with step 0 so PSUM accumulates the fold."""
                t, j, s0, m = pairs[i]
                b0, w = pad[i]
                bank_left[t // 4] -= 1
                off = sel_off[i]
                out = scT[b0 * 16 : (b0 + w) * 16, t, :]
                if nk > 1:
                    out = _bcast_free(out, nk, 0)
                nc.tensor.matmul(
                    out,
                    selp[:, off : off + w * 16],
                    prod[:, poff : poff + nk * 128],
                    start=False,
                    stop=True,
                    skip_group_check=True,
                    tile_position=(0, b0 * 16),
                )

            def emit_score(i, nk):
                """pair i: DVE mul + partial folds, aliased PE score MM."""
                t, j, s0, m = pairs[i]
                prod = prodp.tile([128, KCH * 128], BF16, tag="prod")
                nc.vector.tensor_mul(
                    prod.rearrange("p (k b) -> p k b", b=128),
                    qT[:, :, t * 128 : (t + 1) * 128],
                    kT[:, :, j * 128 : (j + 1) * 128],
                )
                half = KCH * 128 // 2
                while half >= nk * 128:
                    nc.vector.tensor_add(
                        prod[:, 0:half], prod[:, 0:half], prod[:, half : 2 * half]
                    )
                    half //= 2
                _score_mm(i, prod, 0, nk)

            def _ins_free(ap, step, n, pos):
                new = list(ap.ap)
                new.insert(1 + pos, [step, n])
                return bass.AP(tensor=ap.tensor, offset=ap.offset, ap=new)

            def emit_score2(i1, i2, nk):
                """two pairs of the same table t, j1 < j2: batched DVE ops."""
                t, j1, _, _ = pairs[i1]
                _, j2, _, _ = pairs[i2]
                prod = prodp.tile([128, 2 * KCH * 128], BF16, tag="prod")
                pv = prod.rearrange("p (u k b) -> p u k b", b=128, k=KCH)
                qv = qT[:, :, t * 128 : (t + 1) * 128]
                kb = _ins_free(
                    kT[:, :, j1 * 128 : (j1 + 1) * 128], (j2 - j1) * 128, 2, 0
                )
                nc.vector.tensor_mul(pv, _bcast_free(qv, 2, 0), kb)
                n = KCH // 2
                while n >= nk:
                    nc.vector.tensor_add(
                        pv[:, :, 0:n, :], pv[:, :, 0:n, :], pv[:, :, n : 2 * n, :]
                    )
                    n //= 2
                _score_mm(i1, prod, 0, nk)
                _score_mm(i2, prod, KCH * 128, nk)

            def emit_bank_chain(bk):
                """bank bk score-complete: exp, Z (from E, pre-rw), rw mul,
                1/Z, replicate -- all batched across the bank's 4 tables.

                The exp covers the whole PSUM bank region so its dependency
                spans all four tables' matmul writes (reading one table's
                quarter while a sibling's matmul still writes the same bank
                corrupts the read). The Z matmul reads E before the in-place
                rw multiply (WAR dep orders them)."""
                t0 = 4 * bk
                nc.scalar.activation(
                    Pt[:, t0 : t0 + 4, :], scT[:, t0 : t0 + 4, :], AF.Exp
                )
                ch = chainps.tile([128, 512], F32, tag="ch", name="ch")
                zp = ch[0:16, :]
                nc.tensor.matmul(
                    zp,
                    ones_sel,
                    Pt[:, t0 : t0 + 4, :],
                    start=True,
                    stop=True,
                )
                rwb = bass.AP(
                    tensor=rwrep.tensor,
                    offset=rwrep[:, t0 : t0 + 4].offset,
                    ap=list(rwrep.ap[:1]) + [[1, 4], [0, 128]],
                )
                nc.vector.tensor_mul(
                    Pt[:, t0 : t0 + 4, :], Pt[:, t0 : t0 + 4, :], rwb
                )
                zs = zsbp.tile([16, 512], F32, tag="zs")
                nc.scalar.copy(out=zs, in_=zp)
                rz = zsbp.tile([16, 512], F32, tag="rz")
                nc.vector.reciprocal_approx_fast(out=rz, in_=zs)
                wt = workps.tile([128, QW], F32, tag="pp", name="rp")
                rp = wt[:]
                nc.tensor.matmul(rp, replpat, rz, start=True, stop=True)
                nc.scalar.copy(out=rzrepS[:, t0 : t0 + 4, :], in_=rp)

            ctx_first = set()
            pbs_of = {}

            def emit_merge(ci, t, s0, m):
                """pair's merged/broadcast P~ -> SBUF pbs (no v dependency)."""
                idx = merge_idx[(s0, m)]
                wt = workps.tile([128, QW], F32, tag="pp", name="pb")
                pb = wt[:, 0:128]
                nc.tensor.matmul(
                    pb,
                    mergeT[:, idx * 128 : (idx + 1) * 128],
                    Pt[:, t, :],
                    start=True,
                    stop=True,
                )
                pbs = pbsbp.tile([128, 128], BF16, tag="pbs")
                nc.scalar.copy(out=pbs, in_=pb)
                pbs_of[ci] = pbs

            def emit_ctx(ci, t, j):
                """ctx[t] += pbs * vT[j] (into qT's block)."""
                eng = nc.vector
                pbs = pbs_of.pop(ci)
                vs = vT[:, :, j * 128 : (j + 1) * 128]
                dst = qT[:, :, t * 128 : (t + 1) * 128]
                if t not in ctx_first:
                    ctx_first.add(t)
                    eng.tensor_mul(dst, vs, _bcast_free(pbs[:], KCH, 0))
                else:
                    tmp = prodp.tile([128, KCH * 128], BF16, tag="prod")
                    tv = tmp.rearrange("p (k b) -> p k b", b=128)
                    eng.tensor_mul(tv, vs, _bcast_free(pbs[:], KCH, 0))
                    eng.tensor_add(dst, dst, tv)

            def emit_ctx_norm_bank(bk):
                """one batched 1/Z multiply across the bank's 4 t-blocks."""
                t0 = 4 * bk
                dst = qT[:, :, t0 * 128 : (t0 + 4) * 128]
                rz = bass.AP(
                    tensor=rzrepS.tensor,
                    offset=rzrepS[:, t0, :].offset,
                    ap=list(rzrepS.ap[:1]) + [[0, KCH], [1, 4 * 128]],
                )
                nc.vector.tensor_mul(dst, dst, rz)

            def emit_o_costep(wo_t, g, co):
                ps = workps.tile([128, 512], F32, tag="pp", name="pp")
                for ci in range(KCH):
                    nc.tensor.matmul(
                        ps,
                        wo_t[:, ci, co * 128 : (co + 1) * 128],
                        qT[:, ci, g * 512 : (g + 1) * 512],
                        start=(ci == 0),
                        stop=(ci == KCH - 1),
                    )
                ob = outp.tile([128, 512], F32)
                if use_bo:
                    nc.scalar.activation(ob, ps, AF.Identity, bias=boe[:, co])
                else:
                    nc.scalar.copy(out=ob, in_=ps)
                nc.sync.dma_start(
                    out=out_ext[co, :, g * 512 : (g + 1) * 512], in_=ob
                )

            # ---------- wave scheduler ----------
            q_qdone = [False] * NQ
            k_qdone = [False] * NQ
            v_qdone = [False] * NQ
            todo_pairs = list(range(len(pairs)))
            todo_merge = []  # pair indices awaiting merge (pbs) emission
            todo_ctx = []  # pair indices, filled per table as chains emit
            ctx_left = {t: 0 for t in range(T)}
            for (t, j, s0, m) in pairs:
                ctx_left[t] += 1
            chain_done = set()
            table_done = set()

            def emit_scores_ready(budget):
                """emit up to budget ready score pairs, 2-batched per table."""
                n = 0
                ready = [
                    pi
                    for pi in todo_pairs
                    if q_qdone[pairs[pi][0] // 4] and k_qdone[pairs[pi][1] // 4]
                ]
                by_t = {}
                for pi in ready:
                    by_t.setdefault(pairs[pi][0], []).append(pi)
                for t, pis in sorted(by_t.items()):
                    if n >= budget:
                        break
                    nk = 4 if k_qdone[3] else 1
                    pis.sort(key=lambda pi: pairs[pi][1])
                    while len(pis) >= 2 and n + 2 <= budget:
                        i1, i2 = pis.pop(0), pis.pop(0)
                        todo_pairs.remove(i1)
                        todo_pairs.remove(i2)
                        emit_score2(i1, i2, nk)
                        n += 2
                    if pis and n < budget:
                        pi = pis.pop(0)
                        todo_pairs.remove(pi)
                        emit_score(pi, nk)
                        n += 1
                # bank completions -> exp + chains
                for bk in range(4):
                    if bank_left[bk] == 0 and (4 * bk) not in chain_done:
                        emit_bank_chain(bk)
                        for t2 in range(4 * bk, 4 * bk + 4):
                            chain_done.add(t2)
                            for ci2, (t3, _, _, _) in enumerate(pairs):
                                if t3 == t2:
                                    todo_merge.append(ci2)
                                    todo_ctx.append(ci2)
                return n

            def flush(budget):
                n = emit_scores_ready(budget)
                # merges: v-independent, bounded by the pbs pool depth
                while todo_merge and len(pbs_of) < 12:
                    ci = todo_merge.pop(0)
                    t, j, s0, m = pairs[ci]
                    emit_merge(ci, t, s0, m)
                i = 0
                while i < len(todo_ctx) and n < budget:
                    ci2 = todo_ctx[i]
                    t, j, s0, m = pairs[ci2]
                    if ci2 in pbs_of and v_qdone[j // 4]:
                        todo_ctx.pop(i)
                        emit_ctx(ci2, t, j)
                        n += 1
                        ctx_left[t] -= 1
                        if ctx_left[t] == 0:
                            table_done.add(t)
                            bk2 = t // 4
                            if all(
                                ctx_left[t2] == 0
                                for t2 in range(4 * bk2, 4 * bk2 + 4)
                            ) and all(
                                t2 in chain_done
                                for t2 in range(4 * bk2, 4 * bk2 + 4)
                            ):
                                emit_ctx_norm_bank(bk2)
                    else:
                        i += 1
                return n

            # ---------- main schedule ----------
            # v0/v1 pulled before k3 so ctx is not v-gated at the tail; wq
            # dies at q3 so wv rotates into its slot (2 weight slots total)
            phases = [
                ("q", 0), ("k", 0), ("q", 1), ("k", 1), ("q", 2), ("k", 2),
                ("q", 3), ("k", 3), ("v", 0), ("v", 1), ("v", 2), ("v", 3),
            ]
            wv_t = wo_t = None
            for (pn, qtr) in phases:
                w_t = {"q": wq_t, "k": wk_t, "v": wv_t}[pn]
                dst = {"q": qT, "k": kT, "v": vT}[pn]
                bias_t = bqp if (pn == "q" and use_bq) else None
                early = (pn, qtr) in (("q", 0), ("k", 0))
                for c in range(KCH):
                    proj_qstep(w_t, dst, qtr, c, bias_t, dve_copy=early)
                    flush(2 if pn != "v" else 3)
                {"q": q_qdone, "k": k_qdone, "v": v_qdone}[pn][qtr] = True
                flush(2)
                if pn == "q" and qtr == 3:
                    # wq dead -> prefetch wv into its slot (overlaps k3)
                    wv_t = wpool.tile([128, KCH, D], BF16, tag="w")
                    nc.gpsimd.dma_start(
                        out=wv_t, in_=wv_ext.rearrange("(k p) o -> p k o", p=128)
                    )
                if pn == "k" and qtr == 3:
                    # wk dead -> prefetch wo into its slot (overlaps v)
                    wo_t = wpool.tile([128, KCH, D], BF16, tag="w")
                    nc.gpsimd.dma_start(
                        out=wo_t, in_=wo_ext.rearrange("(k p) o -> p k o", p=128)
                    )

            # drain all remaining score/merge/ctx emission (execution is
            # semaphore-paced), then pre-emit o-groups in bank order: their
            # matmuls wait on the norm semaphores, keeping the PE FIFO fed
            guard = 0
            while todo_pairs or todo_ctx or todo_merge:
                made = flush(8)
                guard += 1
                if made == 0 and not todo_merge and guard > 8000:
                    raise RuntimeError(
                        f"stuck: {[(t, ctx_left[t]) for t in range(T)]}"
                    )
            for g in range(4):
                for co in range(KCH):
                    emit_o_costep(wo_t, g, co)

    return nc


_CACHE = {}


def _get_program(rel_idx, use_bq, use_bo):
    key = (rel_idx.tobytes(), use_bq, use_bo)
    if key not in _CACHE:
        nc = _build(rel_idx, use_bq, use_bo)
        nc.finalize()
        _CACHE[key] = nc
    return _CACHE[key]


def kernel(
    table_embs,
    rel_embs,
    rel_idx,
    Wq,
    bq,
    Wk,
    bk,
    Wv,
    bv,
    Wo,
    bo,
    w_rel,
    b_rel,
    _trace=False,
):
    table_embs = np.asarray(table_embs, dtype=np.float32)
    rel_embs = np.asarray(rel_embs, dtype=np.float32)
    rel_idx = np.asarray(rel_idx).astype(np.int64)
    Wq, Wk, Wv, Wo = (np.asarray(w, dtype=np.float32) for w in (Wq, Wk, Wv, Wo))
    bq, bk, bv, bo = (np.asarray(b, dtype=np.float32) for b in (bq, bk, bv, bo))
    w_rel = np.asarray(w_rel, dtype=np.float32)
    b_rel = np.asarray(b_rel, dtype=np.float32)

    pairs, slot_r, merge_idx = _structure(rel_idx)
    ncmb = len(merge_idx)

    # ---- host-side prep ----
    rw = 1.0 / (1.0 + np.exp(-(rel_embs @ w_rel + b_rel[0])))  # [T, R] fp32
    bf = ml_dtypes.bfloat16
    wq_p = np.ascontiguousarray(Wq.T[:, _PERM], dtype=bf)
    wk_p = np.ascontiguousarray(Wk.T[:, _PERM], dtype=bf)
    wv_p = np.ascontiguousarray(Wv.T[:, _PERM], dtype=bf)
    wo_p = np.ascontiguousarray(Wo.T[_PERM, :], dtype=bf)

    rw_slot = np.take_along_axis(rw, slot_r, axis=1)  # [T, S=R]
    pmod = np.arange(128) % 16
    onehot = (pmod[:, None] == np.arange(16)[None, :]).astype(np.float32)

    # selp: per-pair zero-padded selection stationary [128, w*16]
    blocks = []
    for i, (t, j, s0, m) in enumerate(pairs):
        b0, w = _pad_window(s0, m)
        blk = np.zeros((128, w, 16), np.float32)
        for s in range(s0, s0 + m):
            blk[:, s - b0, :] = 0.125 * rw_slot[t, s] * onehot
        blocks.append(blk.reshape(128, w * 16))
    selp = np.ascontiguousarray(np.concatenate(blocks, axis=1), dtype=bf)

    # ones16[p, h'] = 1{p%16==h'};  rwrep[p, t*128+b] = rw_slot[t, p//16]
    ones16 = np.ascontiguousarray(onehot, dtype=bf)
    rwrep = np.ascontiguousarray(
        rw_slot[:, np.arange(128) // 16].T, dtype=bf
    )  # [128, T]

    # mergeT[p=(s,h), idx*128 + i] = 1{s0<=s<s0+m} * 1{i%16 == h}
    mrg = np.zeros((128, ncmb, 128), np.float32)
    smod = np.arange(128) // 16
    hmod = np.arange(128) % 16
    for (s0, m), idx in merge_idx.items():
        mask = ((smod >= s0) & (smod < s0 + m)).astype(np.float32)
        mrg[:, idx, :] = mask[:, None] * (hmod[:, None] == pmod[None, :])
    mrg = np.ascontiguousarray(mrg.reshape(128, ncmb * 128), dtype=bf)

    # replpat[p<16, col] = 1{col%16 == p}
    repl = np.ascontiguousarray(
        (np.arange(128)[None, :] % 16 == np.arange(16)[:, None]).astype(np.float32)
    )

    use_bq = bool(np.any(bq))
    bo_eff = Wo @ bv + bo
    use_bo = bool(np.any(bo_eff))
    bqp = np.ascontiguousarray(bq[_PERM].reshape(KCH, 128).T, dtype=np.float32)
    boe = np.ascontiguousarray(bo_eff.reshape(KCH, 128).T, dtype=np.float32)

    nc = _get_program(rel_idx, use_bq, use_bo)

    in_maps = []
    for c in range(NCORES):
        e = table_embs[:, c * BC : (c + 1) * BC, :]  # [T, BC, D]
        embT = np.ascontiguousarray(
            e.transpose(2, 0, 1).reshape(KCH, 128, TB), dtype=bf
        )
        m = {
            "emb": embT,
            "wq": wq_p,
            "wk": wk_p,
            "wv": wv_p,
            "wo": wo_p,
            "selp": selp,
            "ones16": ones16,
            "rwrep": rwrep,
            "mrg": mrg,
            "repl": repl,
        }
        if use_bq:
            m["bqp"] = bqp
        if use_bo:
            m["boe"] = boe
        in_maps.append(m)

    res = run_bass_kernel_spmd(nc, in_maps, list(range(NCORES)), trace=_trace)
    out = np.empty((T, B, D), dtype=np.float32)
    for c in range(NCORES):
        o = res.results[c]["out"]  # [KCH, 128, TB]
        out[:, c * BC : (c + 1) * BC, :] = (
            o.reshape(D, T, BC).transpose(1, 2, 0)
        )
    if _trace:
        kernel._last_results = res
    return out


# revision 44
# speedup vs baseline: 1.1314x; 1.1314x over previous
"""CrossTableAttention Trainium2 kernel v4 (8-core SPMD, batch-sharded).

Math (per table t, row b, head h, relation slot s):
  rw[t,r]   = sigmoid(rel_embs[t,r] . w_rel + b_rel)             (host)
  qT[f,tb]  = (emb @ Wq.T).T  (feature-on-partition, dh-major perm)
  kT, vT    likewise (bk softmax-invariant -> dropped; bv folds to host
              output addend Wo@bv since sum_r attn = 1)
  sc[(s,h), b] = 0.125*rw * sum_f qT kT     (DVE mul+folds, then one PE
              matmul per (t,j) pair with zero-padded selection stationary,
              accumulating straight into a per-table PSUM tile scT)
  P~ = exp(sc + ln rw)        (one ACT op per table, bias-folded rw)
  Z[h,b] = sum_s exp(sc)      (tiny PE matmul vs 1/rw selection weights)
  ctx_unnorm[f,b] = sum_pairs (merged P~)(h(f),b) * vT[f,jb]   (DVE)
  ctx = ctx_unnorm * (1/Z)[h(f),b]   (replicated via tiny PE matmul)
  out = (ctx.T @ Wo.T).T

v4 structure (from v3 trace: DVE idle first 90us, PE starved 75us mid-run,
both engines ~70% busy over a 367us span vs ~250us of work each):
 - q/k projections interleaved at column-QUARTER granularity so the first
   score pairs are ready ~40us in; v follows; o fills the PE tail.
 - scores land transposed [(s,h), b] in a per-table PSUM tile built by
   per-pair accumulating matmuls (shared-bank start-flag clears); kills
   the per-table PE transpose and the softmax serialization of v3.
 - softmax: one Exp ACT per table with ln(rw) bias (P~ = rw*e in one op),
   Z via a 16-col matmul against 1/rw selection weights, reciprocal on
   DVE from SBUF, replication to 128 partitions via a tiny PE matmul.
 - ctx accumulates into qT's SBUF block per table (dead exactly then),
   normalized once per table; o-projection per 4-table group streams out
   as soon as its ctx blocks normalize.
"""

import sys

sys.path.insert(0, "/opt/trn_rl_repo")

import numpy as np
import ml_dtypes

import concourse.bass as bass
import concourse.bacc as bacc_mod
import concourse.mybir as mybir
import concourse.tile as tile
from concourse.bass_utils import run_bass_kernel_spmd

T, B, D, R, H = 16, 1024, 1024, 8, 16
DH = D // H  # 64
NCORES = 8
BC = B // NCORES  # 128 rows per core
KCH = D // 128  # 8 feature chunks
TB = T * BC  # 2048 (t,b) columns per core
NQ = 4  # column quarters for q/k
QW = TB // NQ  # 512 cols per quarter

F32 = mybir.dt.float32
BF16 = mybir.dt.bfloat16
AF = mybir.ActivationFunctionType

# feature permutation: new f = dh*16 + h  <->  old o = h*64 + dh
_PERM = np.array([(f % H) * DH + f // H for f in range(D)], dtype=np.int64)


def _bcast_free(ap, n, pos):
    """Insert a [step=0, n] broadcast dim into an AP's free dims at `pos`."""
    new = list(ap.ap)
    new.insert(1 + pos, [0, n])
    return bass.AP(tensor=ap.tensor, offset=ap.offset, ap=new)


def _structure(rel_idx):
    """Slot assignment: per t, unique j's get consecutive slot runs."""
    pairs = []  # (t, j, s0, m)
    slot_r = np.zeros((T, R), np.int64)
    for t in range(T):
        by_j = {}
        for r in range(R):
            by_j.setdefault(int(rel_idx[t, r]), []).append(r)
        s0 = 0
        for j, rs in sorted(by_j.items()):
            pairs.append((t, j, s0, len(rs)))
            for i, r in enumerate(rs):
                slot_r[t, s0 + i] = r
            s0 += len(rs)
    combos = sorted({(s0, m) for (_, _, s0, m) in pairs})
    merge_idx = {c: i for i, c in enumerate(combos)}
    return pairs, slot_r, merge_idx


def _pad_window(s0, m):
    """Smallest (b0, w) slot window covering [s0, s0+m) with a 32-aligned
    partition base and legal tile col size: w=2 -> b0 in {0,2,4,6};
    w=4 -> {0,4}; w>=6 -> b0=0."""
    for w, bases in ((2, (0, 2, 4, 6)), (4, (0, 4)), (6, (0,)), (8, (0,))):
        for b0 in bases:
            if b0 <= s0 and b0 + w >= s0 + m:
                return b0, w
    raise AssertionError((s0, m))


def _build(rel_idx, use_bq, use_bo):
    pairs, slot_r, merge_idx = _structure(rel_idx)
    ncmb = len(merge_idx)
    # selp packing offsets
    sel_off = {}
    off = 0
    pad = {}
    for i, (t, j, s0, m) in enumerate(pairs):
        b0, w = _pad_window(s0, m)
        pad[i] = (b0, w)
        sel_off[i] = off
        off += w * 16
    sel_cols = off

    nc = bacc_mod.Bacc(None, target_bir_lowering=False, debug=False)
    emb_ext = nc.dram_tensor("emb", [KCH, 128, TB], BF16, kind="ExternalInput")
    wq_ext = nc.dram_tensor("wq", [D, D], BF16, kind="ExternalInput")
    wk_ext = nc.dram_tensor("wk", [D, D], BF16, kind="ExternalInput")
    wv_ext = nc.dram_tensor("wv", [D, D], BF16, kind="ExternalInput")
    wo_ext = nc.dram_tensor("wo", [D, D], BF16, kind="ExternalInput")
    selp_ext = nc.dram_tensor("selp", [128, sel_cols], BF16, kind="ExternalInput")
    ones_ext = nc.dram_tensor("ones16", [128, 16], BF16, kind="ExternalInput")
    rwrep_ext = nc.dram_tensor("rwrep", [128, T], BF16, kind="ExternalInput")
    merge_ext = nc.dram_tensor("mrg", [128, ncmb * 128], BF16, kind="ExternalInput")
    repl_ext = nc.dram_tensor("repl", [16, 128], F32, kind="ExternalInput")
    if use_bq:
        bq_ext = nc.dram_tensor("bqp", [128, KCH], F32, kind="ExternalInput")
    if use_bo:
        bo_ext = nc.dram_tensor("boe", [128, KCH], F32, kind="ExternalInput")
    out_ext = nc.dram_tensor("out", [KCH, 128, TB], F32, kind="ExternalOutput")

    with tile.TileContext(nc) as tc:
        with (
            # PSUM budget (8 banks): scT 4 + work 3 + chain 1
            tc.tile_pool(name="scps", bufs=1, space="PSUM") as scps,
            tc.tile_pool(name="workps", bufs=3, space="PSUM") as workps,
            tc.tile_pool(name="chainps", bufs=1, space="PSUM") as chainps,
            tc.tile_pool(name="consts", bufs=1) as consts,
            tc.tile_pool(name="wpool", bufs=2) as wpool,
            tc.tile_pool(name="embp", bufs=1) as embp,
            tc.tile_pool(name="qp", bufs=1) as qp,
            tc.tile_pool(name="kp", bufs=1) as kp,
            tc.tile_pool(name="vp", bufs=1) as vp,
            tc.tile_pool(name="pt", bufs=1) as ptp,
            tc.tile_pool(name="prodp", bufs=3) as prodp,
            tc.tile_pool(name="pbsb", bufs=16) as pbsbp,
            tc.tile_pool(name="zsb", bufs=1) as zsbp,
            tc.tile_pool(name="outp", bufs=2) as outp,
        ):
            # ---- PSUM tiles ----
            scT = scps.tile([128, T, 128], F32)  # 4 banks, per-table scores
            # Pre-zero scT: score matmuls then accumulate with start=False
            # (value-correct for both stale has_written states). A start=True
            # bank-clear MM is unusable here: the scheduler may reorder it
            # after sibling tables' disjoint-region MMs, wiping them.
            nc.vector.memset(scT[:], 0.0)
            # ---- weights first (gate the first csteps), consts off-queue ----
            wq_t = wpool.tile([128, KCH, D], BF16, tag="w")
            nc.gpsimd.dma_start(
                out=wq_t, in_=wq_ext.rearrange("(k p) o -> p k o", p=128)
            )
            wk_t = wpool.tile([128, KCH, D], BF16, tag="w")
            nc.gpsimd.dma_start(
                out=wk_t, in_=wk_ext.rearrange("(k p) o -> p k o", p=128)
            )

            # embT[p, k, tb] — host pre-transposed; quarter-ordered loads
            embT = embp.tile([128, KCH, TB], BF16)
            for qtr in range(NQ):
                for k in range(KCH):
                    eng = nc.sync
                    eng.dma_start(
                        out=embT[:, k, qtr * QW : (qtr + 1) * QW],
                        in_=emb_ext[k, :, qtr * QW : (qtr + 1) * QW],
                    )

            selp = consts.tile([128, sel_cols], BF16)
            nc.gpsimd.dma_start(out=selp, in_=selp_ext[:])
            ones_sel = consts.tile([128, 16], BF16)
            nc.gpsimd.dma_start(out=ones_sel, in_=ones_ext[:])
            rwrep = consts.tile([128, T], BF16)
            nc.gpsimd.dma_start(out=rwrep, in_=rwrep_ext[:])
            mergeT = consts.tile([128, ncmb * 128], BF16)
            nc.gpsimd.dma_start(out=mergeT, in_=merge_ext[:])
            replpat = consts.tile([16, 128], F32)
            nc.gpsimd.dma_start(out=replpat, in_=repl_ext[:])
            if use_bq:
                bqp = consts.tile([128, KCH], F32)
                nc.gpsimd.dma_start(out=bqp, in_=bq_ext[:])
            if use_bo:
                boe = consts.tile([128, KCH], F32)
                nc.gpsimd.dma_start(out=boe, in_=bo_ext[:])

            qT = qp.tile([128, KCH, TB], BF16)  # becomes ctx per t-block
            kT = kp.tile([128, KCH, TB], BF16)
            vT = vp.tile([128, KCH, TB], BF16)
            Pt = ptp.tile([128, T, 128], BF16)  # P~ per table
            rzrepS = ptp.tile([128, T, 128], BF16)  # 1/Z replicated per table

            # ---------- emit helpers ----------
            def proj_qstep(w_t, dst_all, qtr, c, bias_t, dve_copy=False):
                """quarter cstep: 8 k-matmuls (N=512) + PSUM->SBUF copy."""
                ps = workps.tile([128, QW], F32, tag="pp", name="pp")
                base = qtr * QW
                for k in range(KCH):
                    nc.tensor.matmul(
                        ps,
                        w_t[:, k, c * 128 : (c + 1) * 128],
                        embT[:, k, base : base + QW],
                        start=(k == 0),
                        stop=(k == KCH - 1),
                    )
                d = dst_all[:, c, base : base + QW]
                if bias_t is not None:
                    nc.scalar.activation(d, ps, AF.Identity, bias=bias_t[:, c])
                elif dve_copy:
                    nc.vector.tensor_copy(out=d, in_=ps)
                else:
                    nc.scalar.copy(out=d, in_=ps)

            bank_left = [0] * 4
            for (t, j, s0, m) in pairs:
                bank_left[t // 4] += 1

            def _score_mm(i, prod, poff, nk):
                """score matmul; moving operand covers nk 128-col chunks of
                the (partially folded) product; the output AP aliases the
                chunk dim with step 0 so PSUM accumulates the fold."""
                t, j, s0, m = pairs[i]
                b0, w = pad[i]
                bank_left[t // 4] -= 1
                off = sel_off[i]
                out = scT[b0 * 16 : (b0 + w) * 16, t, :]
                if nk > 1:
                    out = _bcast_free(out, nk, 0)
                nc.tensor.matmul(
                    out,
                    selp[:, off : off + w * 16],
                    prod[:, poff : poff + nk * 128],
                    start=False,
                    stop=True,
                    skip_group_check=True,
                    tile_position=(0, b0 * 16),
                )

            def emit_score(i, nk):
                """pair i: DVE mul + partial folds, aliased PE score MM."""
                t, j, s0, m = pairs[i]
                prod = prodp.tile([128, KCH * 128], BF16, tag="prod")
                nc.vector.tensor_mul(
                    prod.rearrange("p (k b) -> p k b", b=128),
                    qT[:, :, t * 128 : (t + 1) * 128],
                    kT[:, :, j * 128 : (j + 1) * 128],
                )
                half = KCH * 128 // 2
                while half >= nk * 128:
                    nc.vector.tensor_add(
                        prod[:, 0:half], prod[:, 0:half], prod[:, half : 2 * half]
                    )
                    half //= 2
                _score_mm(i, prod, 0, nk)

            def _ins_free(ap, step, n, pos):
                new = list(ap.ap)
                new.insert(1 + pos, [step, n])
                return bass.AP(tensor=ap.tensor, offset=ap.offset, ap=new)

            def emit_score2(i1, i2, nk):
                """two pairs of the same table t, j1 < j2: batched DVE ops."""
                t, j1, _, _ = pairs[i1]
                _, j2, _, _ = pairs[i2]
                prod = prodp.tile([128, 2 * KCH * 128], BF16, tag="prod")
                pv = prod.rearrange("p (u k b) -> p u k b", b=128, k=KCH)
                qv = qT[:, :, t * 128 : (t + 1) * 128]
                kb = _ins_free(
                    kT[:, :, j1 * 128 : (j1 + 1) * 128], (j2 - j1) * 128, 2, 0
                )
                nc.vector.tensor_mul(pv, _bcast_free(qv, 2, 0), kb)
                n = KCH // 2
                while n >= nk:
                    nc.vector.tensor_add(
                        pv[:, :, 0:n, :], pv[:, :, 0:n, :], pv[:, :, n : 2 * n, :]
                    )
                    n //= 2
                _score_mm(i1, prod, 0, nk)
                _score_mm(i2, prod, KCH * 128, nk)

            def emit_bank_chain(bk):
                """bank bk score-complete: exp, Z (from E, pre-rw), rw mul,
                1/Z, replicate -- all batched across the bank's 4 tables.

                The exp covers the whole PSUM bank region so its dependency
                spans all four tables' matmul writes (reading one table's
                quarter while a sibling's matmul still writes the same bank
                corrupts the read). The Z matmul reads E before the in-place
                rw multiply (WAR dep orders them)."""
                t0 = 4 * bk
                nc.scalar.activation(
                    Pt[:, t0 : t0 + 4, :], scT[:, t0 : t0 + 4, :], AF.Exp
                )
                ch = chainps.tile([128, 512], F32, tag="ch", name="ch")
                zp = ch[0:16, :]
                nc.tensor.matmul(
                    zp,
                    ones_sel,
                    Pt[:, t0 : t0 + 4, :],
                    start=True,
                    stop=True,
                )
                rwb = bass.AP(
                    tensor=rwrep.tensor,
                    offset=rwrep[:, t0 : t0 + 4].offset,
                    ap=list(rwrep.ap[:1]) + [[1, 4], [0, 128]],
                )
                nc.vector.tensor_mul(
                    Pt[:, t0 : t0 + 4, :], Pt[:, t0 : t0 + 4, :], rwb
                )
                zs = zsbp.tile([16, 512], F32, tag="zs")
                nc.scalar.copy(out=zs, in_=zp)
                rz = zsbp.tile([16, 512], F32, tag="rz")
                nc.vector.reciprocal_approx_fast(out=rz, in_=zs)
                wt = workps.tile([128, QW], F32, tag="pp", name="rp")
                rp = wt[:]
                nc.tensor.matmul(rp, replpat, rz, start=True, stop=True)
                nc.scalar.copy(out=rzrepS[:, t0 : t0 + 4, :], in_=rp)

            ctx_first = set()
            pbs_of = {}

            def emit_merge(ci, t, s0, m):
                """pair's merged/broadcast P~ -> SBUF pbs (no v dependency)."""
                idx = merge_idx[(s0, m)]
                wt = workps.tile([128, QW], F32, tag="pp", name="pb")
                pb = wt[:, 0:128]
                nc.tensor.matmul(
                    pb,
                    mergeT[:, idx * 128 : (idx + 1) * 128],
                    Pt[:, t, :],
                    start=True,
                    stop=True,
                )
                pbs = pbsbp.tile([128, 128], BF16, tag="pbs")
                nc.scalar.copy(out=pbs, in_=pb)
                pbs_of[ci] = pbs

            def emit_ctx(ci, t, j):
                """ctx[t] += pbs * vT[j] (into qT's block)."""
                eng = nc.vector
                pbs = pbs_of.pop(ci)
                vs = vT[:, :, j * 128 : (j + 1) * 128]
                dst = qT[:, :, t * 128 : (t + 1) * 128]
                if t not in ctx_first:
                    ctx_first.add(t)
                    eng.tensor_mul(dst, vs, _bcast_free(pbs[:], KCH, 0))
                else:
                    tmp = prodp.tile([128, KCH * 128], BF16, tag="prod")
                    tv = tmp.rearrange("p (k b) -> p k b", b=128)
                    eng.tensor_mul(tv, vs, _bcast_free(pbs[:], KCH, 0))
                    eng.tensor_add(dst, dst, tv)

            def emit_ctx_norm_bank(bk):
                """one batched 1/Z multiply across the bank's 4 t-blocks."""
                t0 = 4 * bk
                dst = qT[:, :, t0 * 128 : (t0 + 4) * 128]
                rz = bass.AP(
                    tensor=rzrepS.tensor,
                    offset=rzrepS[:, t0, :].offset,
                    ap=list(rzrepS.ap[:1]) + [[0, KCH], [1, 4 * 128]],
                )
                nc.vector.tensor_mul(dst, dst, rz)

            def emit_o_costep(wo_t, g, co):
                ps = workps.tile([128, 512], F32, tag="pp", name="pp")
                for ci in range(KCH):
                    nc.tensor.matmul(
                        ps,
                        wo_t[:, ci, co * 128 : (co + 1) * 128],
                        qT[:, ci, g * 512 : (g + 1) * 512],
                        start=(ci == 0),
                        stop=(ci == KCH - 1),
                    )
                ob = outp.tile([128, 512], F32)
                if use_bo:
                    nc.scalar.activation(ob, ps, AF.Identity, bias=boe[:, co])
                else:
                    nc.scalar.copy(out=ob, in_=ps)
                nc.sync.dma_start(
                    out=out_ext[co, :, g * 512 : (g + 1) * 512], in_=ob
                )

            # ---------- wave scheduler ----------
            q_qdone = [False] * NQ
            k_qdone = [False] * NQ
            v_qdone = [False] * NQ
            todo_pairs = list(range(len(pairs)))
            todo_merge = []  # pair indices awaiting merge (pbs) emission
            todo_ctx = []  # pair indices, filled per table as chains emit
            ctx_left = {t: 0 for t in range(T)}
            for (t, j, s0, m) in pairs:
                ctx_left[t] += 1
            chain_done = set()
            table_done = set()

            def emit_scores_ready(budget):
                """emit up to budget ready score pairs, 2-batched per table."""
                n = 0
                ready = [
                    pi
                    for pi in todo_pairs
                    if q_qdone[pairs[pi][0] // 4] and k_qdone[pairs[pi][1] // 4]
                ]
                by_t = {}
                for pi in ready:
                    by_t.setdefault(pairs[pi][0], []).append(pi)
                for t, pis in sorted(by_t.items()):
                    if n >= budget:
                        break
                    nk = 4 if k_qdone[3] else 2
                    pis.sort(key=lambda pi: pairs[pi][1])
                    while len(pis) >= 2 and n + 2 <= budget:
                        i1, i2 = pis.pop(0), pis.pop(0)
                        todo_pairs.remove(i1)
                        todo_pairs.remove(i2)
                        emit_score2(i1, i2, nk)
                        n += 2
                    if pis and n < budget:
                        pi = pis.pop(0)
                        todo_pairs.remove(pi)
                        emit_score(pi, nk)
                        n += 1
                # bank completions -> exp + chains
                for bk in range(4):
                    if bank_left[bk] == 0 and (4 * bk) not in chain_done:
                        emit_bank_chain(bk)
                        for t2 in range(4 * bk, 4 * bk + 4):
                            chain_done.add(t2)
                            for ci2, (t3, _, _, _) in enumerate(pairs):
                                if t3 == t2:
                                    todo_merge.append(ci2)
                                    todo_ctx.append(ci2)
                return n

            def flush(budget):
                n = emit_scores_ready(budget)
                # merges: v-independent, bounded by the pbs pool depth
                while todo_merge and len(pbs_of) < 12:
                    ci = todo_merge.pop(0)
                    t, j, s0, m = pairs[ci]
                    emit_merge(ci, t, s0, m)
                i = 0
                while i < len(todo_ctx) and n < budget:
                    ci2 = todo_ctx[i]
                    t, j, s0, m = pairs[ci2]
                    if ci2 in pbs_of and v_qdone[j // 4]:
                        todo_ctx.pop(i)
                        emit_ctx(ci2, t, j)
                        n += 1
                        ctx_left[t] -= 1
                        if ctx_left[t] == 0:
                            table_done.add(t)
                            bk2 = t // 4
                            if all(
                                ctx_left[t2] == 0
                                for t2 in range(4 * bk2, 4 * bk2 + 4)
                            ) and all(
                                t2 in chain_done
                                for t2 in range(4 * bk2, 4 * bk2 + 4)
                            ):
                                emit_ctx_norm_bank(bk2)
                    else:
                        i += 1
                return n

            # ---------- main schedule ----------
            # v0/v1 pulled before k3 so ctx is not v-gated at the tail; wq
            # dies at q3 so wv rotates into its slot (2 weight slots total)
            phases = [
                ("q", 0), ("k", 0), ("q", 1), ("k", 1), ("q", 2), ("k", 2),
                ("q", 3), ("k", 3), ("v", 0), ("v", 1), ("v", 2), ("v", 3),
            ]
            wv_t = wo_t = None
            for (pn, qtr) in phases:
                w_t = {"q": wq_t, "k": wk_t, "v": wv_t}[pn]
                dst = {"q": qT, "k": kT, "v": vT}[pn]
                bias_t = bqp if (pn == "q" and use_bq) else None
                early = (pn, qtr) in (("q", 0), ("k", 0))
                for c in range(KCH):
                    proj_qstep(w_t, dst, qtr, c, bias_t, dve_copy=early)
                    flush(2 if pn != "v" else 3)
                {"q": q_qdone, "k": k_qdone, "v": v_qdone}[pn][qtr] = True
                flush(2)
                if pn == "q" and qtr == 3:
                    # wq dead -> prefetch wv into its slot (overlaps k3)
                    wv_t = wpool.tile([128, KCH, D], BF16, tag="w")
                    nc.gpsimd.dma_start(
                        out=wv_t, in_=wv_ext.rearrange("(k p) o -> p k o", p=128)
                    )
                if pn == "k" and qtr == 3:
                    # wk dead -> prefetch wo into its slot (overlaps v)
                    wo_t = wpool.tile([128, KCH, D], BF16, tag="w")
                    nc.gpsimd.dma_start(
                        out=wo_t, in_=wo_ext.rearrange("(k p) o -> p k o", p=128)
                    )

            # o-projection per 4-table group, in ctx-completion order
            o_done = set()
            guard = 0
            while len(o_done) < 4:
                g = next(
                    (
                        g
                        for g in range(4)
                        if g not in o_done
                        and all(t in table_done for t in range(4 * g, 4 * g + 4))
                    ),
                    None,
                )
                if g is None:
                    made = flush(6)
                    guard += 1
                    if made == 0 and not todo_merge and guard > 8000:
                        raise RuntimeError(
                            f"stuck: {[(t, ctx_left[t]) for t in range(T)]}"
                        )
                    continue
                for co in range(KCH):
                    emit_o_costep(wo_t, g, co)
                    flush(2)
                o_done.add(g)
            while todo_pairs or todo_ctx or todo_merge:
                if flush(8) == 0 and not todo_merge:
                    raise RuntimeError("scheduler stuck at tail")

    return nc


_CACHE = {}


def _get_program(rel_idx, use_bq, use_bo):
    key = (rel_idx.tobytes(), use_bq, use_bo)
    if key not in _CACHE:
        nc = _build(rel_idx, use_bq, use_bo)
        nc.finalize()
        _CACHE[key] = nc
    return _CACHE[key]


def kernel(
    table_embs,
    rel_embs,
    rel_idx,
    Wq,
    bq,
    Wk,
    bk,
    Wv,
    bv,
    Wo,
    bo,
    w_rel,
    b_rel,
    _trace=False,
):
    table_embs = np.asarray(table_embs, dtype=np.float32)
    rel_embs = np.asarray(rel_embs, dtype=np.float32)
    rel_idx = np.asarray(rel_idx).astype(np.int64)
    Wq, Wk, Wv, Wo = (np.asarray(w, dtype=np.float32) for w in (Wq, Wk, Wv, Wo))
    bq, bk, bv, bo = (np.asarray(b, dtype=np.float32) for b in (bq, bk, bv, bo))
    w_rel = np.asarray(w_rel, dtype=np.float32)
    b_rel = np.asarray(b_rel, dtype=np.float32)

    pairs, slot_r, merge_idx = _structure(rel_idx)
    ncmb = len(merge_idx)

    # ---- host-side prep ----
    rw = 1.0 / (1.0 + np.exp(-(rel_embs @ w_rel + b_rel[0])))  # [T, R] fp32
    bf = ml_dtypes.bfloat16
    wq_p = np.ascontiguousarray(Wq.T[:, _PERM], dtype=bf)
    wk_p = np.ascontiguousarray(Wk.T[:, _PERM], dtype=bf)
    wv_p = np.ascontiguousarray(Wv.T[:, _PERM], dtype=bf)
    wo_p = np.ascontiguousarray(Wo.T[_PERM, :], dtype=bf)

    rw_slot = np.take_along_axis(rw, slot_r, axis=1)  # [T, S=R]
    pmod = np.arange(128) % 16
    onehot = (pmod[:, None] == np.arange(16)[None, :]).astype(np.float32)

    # selp: per-pair zero-padded selection stationary [128, w*16]
    blocks = []
    for i, (t, j, s0, m) in enumerate(pairs):
        b0, w = _pad_window(s0, m)
        blk = np.zeros((128, w, 16), np.float32)
        for s in range(s0, s0 + m):
            blk[:, s - b0, :] = 0.125 * rw_slot[t, s] * onehot
        blocks.append(blk.reshape(128, w * 16))
    selp = np.ascontiguousarray(np.concatenate(blocks, axis=1), dtype=bf)

    # ones16[p, h'] = 1{p%16==h'};  rwrep[p, t*128+b] = rw_slot[t, p//16]
    ones16 = np.ascontiguousarray(onehot, dtype=bf)
    rwrep = np.ascontiguousarray(
        rw_slot[:, np.arange(128) // 16].T, dtype=bf
    )  # [128, T]

    # mergeT[p=(s,h), idx*128 + i] = 1{s0<=s<s0+m} * 1{i%16 == h}
    mrg = np.zeros((128, ncmb, 128), np.float32)
    smod = np.arange(128) // 16
    hmod = np.arange(128) % 16
    for (s0, m), idx in merge_idx.items():
        mask = ((smod >= s0) & (smod < s0 + m)).astype(np.float32)
        mrg[:, idx, :] = mask[:, None] * (hmod[:, None] == pmod[None, :])
    mrg = np.ascontiguousarray(mrg.reshape(128, ncmb * 128), dtype=bf)

    # replpat[p<16, col] = 1{col%16 == p}
    repl = np.ascontiguousarray(
        (np.arange(128)[None, :] % 16 == np.arange(16)[:, None]).astype(np.float32)
    )

    use_bq = bool(np.any(bq))
    bo_eff = Wo @ bv + bo
    use_bo = bool(np.any(bo_eff))
    bqp = np.ascontiguousarray(bq[_PERM].reshape(KCH, 128).T, dtype=np.float32)
    boe = np.ascontiguousarray(bo_eff.reshape(KCH, 128).T, dtype=np.float32)

    nc = _get_program(rel_idx, use_bq, use_bo)

    in_maps = []
    for c in range(NCORES):
        e = table_embs[:, c * BC : (c + 1) * BC, :]  # [T, BC, D]
        embT = np.ascontiguousarray(
            e.transpose(2, 0, 1).reshape(KCH, 128, TB), dtype=bf
        )
        m = {
            "emb": embT,
            "wq": wq_p,
            "wk": wk_p,
            "wv": wv_p,
            "wo": wo_p,
            "selp": selp,
            "ones16": ones16,
            "rwrep": rwrep,
            "mrg": mrg,
            "repl": repl,
        }
        if use_bq:
            m["bqp"] = bqp
        if use_bo:
            m["boe"] = boe
        in_maps.append(m)

    res = run_bass_kernel_spmd(nc, in_maps, list(range(NCORES)), trace=_trace)
    out = np.empty((T, B, D), dtype=np.float32)
    for c in range(NCORES):
        o = res.results[c]["out"]  # [KCH, 128, TB]
        out[:, c * BC : (c + 1) * BC, :] = (
            o.reshape(D, T, BC).transpose(1, 2, 0)
        )
    if _trace:
        kernel._last_results = res
    return out
